# revision 24
# baseline (speedup 1.0000x reference)
"""Trainium2 Bass kernel for CrossBandWindowAttention.

Reference computation (per window item b of B_=2048):
    q = (x @ Wq + bq) * scale      -> (64, 96), 6 heads x 16
    k = cross_x @ Wk + bk          -> (64, 96)
    v = cross_x @ Wv + bv          -> (64, 384), 6 heads x 64
    L_h = q_h k_h^T + rpb_bias_h (+ mask_w)
    A = softmax(L, axis=-1)
    out = (concat_h A_h v_h) @ Wp + bp
Sharding: data-parallel over b_ across 8 cores (256 windows each).

Per-core design ("transposed-logits" formulation, v2):
  - x/cross_x loaded natural, PE-transposed to channel-major xt/cxt.
  - Q/K projections emit qT/kT (96 = (head, d), tokens) in f16.
  - Q is scattered into a zero-padded block-diagonal tile q_bd so a
    single K=32 matmul per (head-pair c, window s) produces logits
    TRANSPOSED: lps rows = (s, m key-token), cols = (c, j, n query).
    6 matmuls/pair, 6-way concurrent via tile_position (3 row strips x
    2 col groups). No probability transpose is ever needed.
  - softmax: exp on ScalarE; bias folded as exp(L)*exp(bias^T) on DVE
    (f16, 2x/4x mode); row sums over the key dim (partitions) via ONE
    matmul against a constant block-selector sel128 whose (128, 128)
    structure lands the per-window sums already broadcast across all
    partitions; normalize = single DVE divide. No reduce, no transpose,
    no partition broadcast.
  - AV: 12 (64x64) matmuls, 4-way concurrent (2 row x 2 col groups),
    lhsT = v natural slices (window-major rows co-located with the
    transposed-prob rows), output channel-major = final-proj lhsT.
  - proj: 3 accumulating matmuls against Wp chunks -> (128 tokens, 384).
Engine placement: exp/og/tmp on ScalarE, bias/divide/pl on DVE,
v-copy + some transpose copies on GpSimd (otherwise idle).
Matmul operands f32r (projections) / f16 (attention core).
"""

import os
from contextlib import ExitStack

import numpy as np

import concourse.bass as bass
import concourse.mybir as mybir
import concourse.tile as tile
from concourse import bacc
from concourse.bass_utils import run_bass_kernel_spmd
from concourse.masks import make_identity

F32 = mybir.dt.float32
F32R = mybir.dt.float32r
F16 = mybir.dt.float16

DIM = 96
HEADS = 6
HD = 16  # head dim for q/k
VD = 64  # head dim for v
N = 64  # tokens per window
C = 384
NCORES = 8
B_TOTAL = 2048
GRP = 8  # windows per group (512 tokens)
TOK_G = GRP * N  # 512

EXP = mybir.ActivationFunctionType.Exp
IDENT = mybir.ActivationFunctionType.Identity
MULT = mybir.AluOpType.mult
ADD = mybir.AluOpType.add
DIVIDE = mybir.AluOpType.divide


def _build(nw, use_mask, use_bias):
    """Build the per-core Bass module for `nw` windows."""
    nc = bacc.Bacc("TRN2", target_bir_lowering=False, debug=False)

    d_x = nc.dram_tensor("x", [nw, N, C], F32, kind="ExternalInput").ap()
    d_cx = nc.dram_tensor("cx", [nw, N, C], F32, kind="ExternalInput").ap()
    d_wq = nc.dram_tensor("wq", [C, DIM], F32R, kind="ExternalInput").ap()
    d_wk = nc.dram_tensor("wk", [C, DIM], F32R, kind="ExternalInput").ap()
    d_wv = nc.dram_tensor("wv", [C, C], F32R, kind="ExternalInput").ap()
    d_wp = nc.dram_tensor("wp", [C, C], F32R, kind="ExternalInput").ap()
    d_biasT2 = nc.dram_tensor("biasT2", [128, C], F32, kind="ExternalInput").ap()
    d_sel = nc.dram_tensor("sel128", [128, 128], F16, kind="ExternalInput").ap()
    if use_bias:
        d_bq = nc.dram_tensor("bq_c", [DIM, 1], F32, kind="ExternalInput").ap()
        d_bk = nc.dram_tensor("bk_c", [DIM, 1], F32, kind="ExternalInput").ap()
        d_bv2 = nc.dram_tensor("bv2", [128, C], F32, kind="ExternalInput").ap()
        d_bp2 = nc.dram_tensor("bp2", [128, C], F32, kind="ExternalInput").ap()
    if use_mask:
        d_maskT2 = nc.dram_tensor(
            "maskT2", [nw // 2, 128, C], F32, kind="ExternalInput"
        ).ap()
    d_y = nc.dram_tensor("y", [nw, N, C], F32, kind="ExternalOutput").ap()

    x_flat = d_x.rearrange("w n c -> (w n) c")
    cx_flat = d_cx.rearrange("w n c -> (w n) c")
    y_flat = d_y.rearrange("w n c -> (w n) c")

    npair = nw // 2
    n_grp = nw // GRP

    with tile.TileContext(nc) as tc, ExitStack() as ctx:
        const = ctx.enter_context(tc.tile_pool(name="const", bufs=1))
        p_nat = ctx.enter_context(tc.tile_pool(name="p_nat", bufs=4))
        p_xt = ctx.enter_context(tc.tile_pool(name="p_xt", bufs=2))
        p_qk = ctx.enter_context(tc.tile_pool(name="p_qk", bufs=2))
        p_v = ctx.enter_context(tc.tile_pool(name="p_v", bufs=5))
        p_sm = ctx.enter_context(tc.tile_pool(name="p_sm", bufs=5))
        p_pl = ctx.enter_context(tc.tile_pool(name="p_pl", bufs=3))
        p_out = ctx.enter_context(tc.tile_pool(name="p_out", bufs=2))
        # PSUM: 4 pools x 2 bufs = 8 banks exactly.
        ps_a = ctx.enter_context(tc.tile_pool(name="ps_a", bufs=2, space="PSUM"))
        ps_vf = ctx.enter_context(tc.tile_pool(name="ps_vf", bufs=2, space="PSUM"))
        ps_l = ctx.enter_context(tc.tile_pool(name="ps_l", bufs=2, space="PSUM"))
        ps_p = ctx.enter_context(tc.tile_pool(name="ps_p", bufs=2, space="PSUM"))

        # ---- constants in SBUF ----
        ident = const.tile([128, 128], F32, name="ident")
        make_identity(nc, ident[:])

        wq_sb = const.tile([128, 3, DIM], F32R, name="wq_sb")
        wk_sb = const.tile([128, 3, DIM], F32R, name="wk_sb")
        wv_sb = const.tile([128, 3, C], F32R, name="wv_sb")
        wp_sb = const.tile([128, 3, C], F32R, name="wp_sb")
        biasT2_sb = const.tile([128, C], F32, name="biasT2_sb")
        sel_sb = const.tile([128, 128], F16, name="sel_sb")
        for Ci in range(3):
            sl = slice(128 * Ci, 128 * Ci + 128)
            nc.sync.dma_start(wq_sb[:, Ci], d_wq[sl, :])
            nc.sync.dma_start(wk_sb[:, Ci], d_wk[sl, :])
            nc.sync.dma_start(wv_sb[:, Ci], d_wv[sl, :])
            nc.sync.dma_start(wp_sb[:, Ci], d_wp[sl, :])
        nc.sync.dma_start(biasT2_sb[:], d_biasT2[:])
        nc.sync.dma_start(sel_sb[:], d_sel[:])
        expbT_sb = const.tile([128, C], F16, name="expbT_sb")
        nc.scalar.activation(expbT_sb[:], biasT2_sb[:], EXP)
        if use_bias:
            bq_sb = const.tile([DIM, 1], F32, name="bq_sb")
            bk_sb = const.tile([DIM, 1], F32, name="bk_sb")
            bv2_sb = const.tile([128, C], F32, name="bv2_sb")
            bp2_sb = const.tile([128, C], F32, name="bp2_sb")
            nc.sync.dma_start(bq_sb[:], d_bq[:])
            nc.sync.dma_start(bk_sb[:], d_bk[:])
            nc.sync.dma_start(bv2_sb[:], d_bv2[:])
            nc.sync.dma_start(bp2_sb[:], d_bp2[:])

        # q_bd: (32, 3, GRP, 128) f16, block-diagonal per head pair, all
        # chunks at partitions 0-31. Rows 16j+d = head 2c+j of chunk c;
        # cols (w, 64j + n). Two persistent tensors alternating by group
        # parity (a single one would serialize group g+1's build behind
        # group g's QK reads): off-diagonal blocks zeroed once; each
        # group's DMAs rewrite only the diagonal blocks.
        q_bds = []
        for b in range(2):
            t = const.tile([32, 3, GRP, 128], F16, name=f"q_bd{b}")
            nc.vector.memset(t[:], 0.0)
            q_bds.append(t)

        def transpose_in(src_flat, tok0, tag):
            """Load 512 tokens natural; PE-transpose to (128, 3, 512).

            PSUM tiles are packed channel-chunk-major: tile Ci holds the
            4 token-blocks of chunk Ci, so one (128, 512) copy per chunk
            lands contiguously in xt."""
            xt = p_xt.tile([128, 3, TOK_G], F32R, tag=f"xt_{tag}", name=f"xt_{tag}")
            nat = p_nat.tile([128, 4, C], F32, tag="nat", name=f"nat_{tag}")
            nc.sync.dma_start(
                nat[:],
                src_flat[tok0 : tok0 + TOK_G, :].rearrange("(t p) c -> p t c", p=128),
            )
            for Ci in range(3):
                tp = ps_a.tile([128, TOK_G], F32, tag="ps_a", name=f"tps_{tag}{Ci}")
                for t in range(4):
                    nc.tensor.transpose(
                        tp[:, 128 * t : 128 * (t + 1)],
                        nat[:, t, 128 * Ci : 128 * (Ci + 1)],
                        ident[:],
                    )
                if Ci == 2:
                    nc.scalar.copy(xt[:, Ci], tp[:])
                else:
                    nc.vector.tensor_copy(xt[:, Ci], tp[:])
            return xt

        def preamble(g):
            """Group-level: loads, transposes, Q/K projections, q_bd."""
            tok0 = g * TOK_G
            xt = transpose_in(x_flat, tok0, "x")
            cxt = transpose_in(cx_flat, tok0, "c")

            def qk_proj(src_t, w, b, tag):
                pq = ps_a.tile([DIM, TOK_G], F32, tag="ps_a", name=f"pq_{tag}")
                for Ci in range(3):
                    nc.tensor.matmul(
                        pq[:], w[:, Ci], src_t[:, Ci],
                        start=(Ci == 0), stop=(Ci == 2),
                    )
                return pq

            # tmp_q keeps the (96, T) layout (DMA source only); tmp_k is
            # split per head-pair chunk to partitions 0-31 so every QK
            # matmul sits in row strip 0 — concurrent matmuls must never
            # target the same PSUM partitions from different row groups.
            pq_q = qk_proj(xt, wq_sb, bq_sb if use_bias else None, "q")
            tmp_q = p_qk.tile([DIM, TOK_G], F16, tag="tmp_q", name="tmp_q")
            if use_bias:
                nc.scalar.activation(tmp_q[:], pq_q[:], IDENT, bias=bq_sb[:])
            else:
                nc.scalar.copy(tmp_q[:], pq_q[:])
            pq_k = qk_proj(cxt, wk_sb, bk_sb if use_bias else None, "k")
            tmp_k = p_qk.tile([32, 3, TOK_G], F16, tag="tmp_k", name="tmp_k")
            for cc in range(3):
                src = pq_k[32 * cc : 32 * cc + 32, :]
                if use_bias:
                    nc.scalar.activation(
                        tmp_k[:, cc], src, IDENT,
                        bias=bk_sb[32 * cc : 32 * cc + 32],
                    )
                elif cc == 1:
                    nc.scalar.copy(tmp_k[:, cc], src)
                else:
                    nc.vector.tensor_copy(tmp_k[:, cc], src)

            q_bd = q_bds[g % 2]
            for h in range(HEADS):
                cc, j = divmod(h, 2)
                nc.sync.dma_start(
                    q_bd[16 * j : 16 * j + 16, cc, :, 64 * j : 64 * j + 64],
                    tmp_q[16 * h : 16 * h + 16, :].rearrange(
                        "p (w n) -> p w n", n=64
                    ),
                )
            return cxt, tmp_k, q_bd

        # Software-pipelined flat loop over pairs: stage A (V-proj + QK +
        # softmax head) for pair i runs on the PE before stage B (sums +
        # AV + proj) of pair i-1, hiding the scalar/vector latency between
        # QK and the sums matmul.
        stash = {}  # pair index -> tiles needed by stage B
        group_state = None

        def stage_a(i):
            nonlocal group_state
            g, ip = divmod(i, 4)
            if ip == 0:
                group_state = preamble(g)
            cxt, tmp_k, q_bd = group_state
            ptok = 128 * ip  # pair token offset within group

            # ---- V projection (pair tokens natural) ----
            vps = ps_vf.tile([128, C], F32, tag="ps_vf", name="vps")
            for Ci in range(3):
                nc.tensor.matmul(
                    vps[:], cxt[:, Ci, ptok : ptok + 128], wv_sb[:, Ci],
                    start=(Ci == 0), stop=(Ci == 2),
                )
            # per-window v rebased to partitions 0-63 so every AV matmul
            # sits in row strips 0-1 (concurrent matmuls to the same PSUM
            # partitions must share row groups).
            v2 = p_v.tile([64, 2, C], F16, tag="vnat", name="v2")
            for s in range(2):
                sl = slice(64 * s, 64 * s + 64)
                if use_bias:
                    nc.vector.tensor_tensor(
                        v2[:, s], vps[sl, :], bv2_sb[sl, :], op=ADD
                    )
                else:
                    nc.vector.tensor_copy(v2[:, s], vps[sl, :])

            # ---- QK^T: transposed logits, 6 matmuls (2-way concurrent) ----
            # lps rows = (s, m) key tokens; cols = (c, j, n) = 128c+64j+n.
            lps = ps_l.tile([128, C], F32, tag="ps_l", name="lps")
            for cc in range(3):
                for s in range(2):
                    tok_s = ptok + 64 * s
                    nc.tensor.matmul(
                        lps[64 * s : 64 * s + 64, 128 * cc : 128 * cc + 128],
                        tmp_k[:, cc, tok_s : tok_s + 64],
                        q_bd[:, cc, 2 * ip + s, :],
                        start=True, stop=True,
                        tile_position=(0, 64 * s),
                    )

            # ---- exp + relative-position bias ----
            ee = p_sm.tile([128, C], F16, tag="ee", name="ee")
            if use_mask:
                m_sb = p_sm.tile([128, C], F32, tag="msk", name="m_sb")
                nc.sync.dma_start(m_sb[:], d_maskT2[2 * g + ip])
                e_f32 = p_sm.tile([128, C], F32, tag="ef", name="e_f32")
                nc.vector.tensor_tensor(e_f32[:], lps[:], m_sb[:], op=ADD)
                nc.scalar.activation(ee[:], e_f32[:], EXP)
            else:
                nc.scalar.activation(ee[:], lps[:], EXP)
            E = p_sm.tile([128, C], F16, tag="E", name="E")
            nc.vector.tensor_tensor(E[:], ee[:], expbT_sb[:], op=MULT)
            stash[i] = (v2, E)

        def stage_b1(i):
            v2, E = stash.pop(i)

            # ---- key-dim sums, broadcast across partitions, via PE ----
            sps = ps_l.tile([128, C], F32, tag="ps_l", name="sps")
            nc.tensor.matmul(sps[:], sel_sb[:], E[:], start=True, stop=True)
            rec = p_sm.tile([128, C], F32, tag="rec", name="rec")
            nc.vector.reciprocal_approx_fast(rec[:], sps[:])
            een = p_sm.tile([64, 2, C], F16, tag="een", name="een")
            for s in range(2):
                sl = slice(64 * s, 64 * s + 64)
                nc.vector.tensor_tensor(
                    een[:, s], E[sl, :], rec[sl, :], op=MULT
                )
            stash[(i, "b2")] = (v2, een)

        def stage_b2a(i):
            v2, een = stash.pop((i, "b2"))

            # ---- AV: 12 matmuls, 2-way concurrent, channel-major out ----
            # pps rows = (j, d); cols = (c, s, n) = 128c+64s+n. All
            # operands at base 0 (row strips 0-1); consecutive matmuls
            # alternate output col groups (j) so pairs run concurrently.
            pps = ps_p.tile([128, C], F32, tag="ps_p", name="pps")
            for cc in range(3):
                for s in range(2):
                    for j in range(2):
                        h = 2 * cc + j
                        nc.tensor.matmul(
                            pps[64 * j : 64 * j + 64,
                                128 * cc + 64 * s : 128 * cc + 64 * s + 64],
                            v2[:, s, 64 * h : 64 * h + 64],
                            een[:, s,
                                128 * cc + 64 * j : 128 * cc + 64 * j + 64],
                            start=True, stop=True,
                            tile_position=(0, 64 * j),
                        )
            pl_sb = p_pl.tile([128, C], F32R, tag="pl", name="pl_sb")
            nc.scalar.copy(pl_sb[:], pps[:])
            stash[(i, "b2b")] = pl_sb

        def stage_b2b(i):
            g, ip = divmod(i, 4)
            pl_sb = stash.pop((i, "b2b"))
            if ip == 0:
                og = p_out.tile([128, 4, C], F32, tag="og", name="og")
                stash[("og", g)] = og
            og = stash[("og", g)]

            # ---- output projection ----
            fps = ps_vf.tile([128, C], F32, tag="ps_vf", name="fps")
            for Ci in range(3):
                nc.tensor.matmul(
                    fps[:], pl_sb[:, 128 * Ci : 128 * (Ci + 1)], wp_sb[:, Ci],
                    start=(Ci == 0), stop=(Ci == 2),
                )
            if use_bias:
                nc.scalar.activation(og[:, ip], fps[:], IDENT)
                nc.vector.tensor_tensor(og[:, ip], og[:, ip], bp2_sb[:], op=ADD)
            else:
                nc.scalar.copy(og[:, ip], fps[:])
            if ip == 3:
                stash.pop(("og", g))
                tok0 = g * TOK_G
                nc.scalar.dma_start(
                    y_flat[tok0 : tok0 + TOK_G, :].rearrange(
                        "(t p) c -> p t c", p=128
                    ),
                    og[:],
                )

        for i in range(npair):
            stage_a(i)
            if i >= 1:
                stage_b1(i - 1)
            if i >= 2:
                stage_b2a(i - 2)
            if i >= 3:
                stage_b2b(i - 3)
        stage_b1(npair - 1)
        stage_b2a(npair - 2)
        stage_b2b(npair - 3)
        stage_b2a(npair - 1)
        stage_b2b(npair - 2)
        stage_b2b(npair - 1)

    nc.compile()
    return nc


def _prep_host(Wq, bq, Wk, bk, Wv, bv, Wp, bp, rpi, rpb_table, mask):
    scale = HD ** (-0.5)
    Wq = np.asarray(Wq, dtype=np.float32) * scale
    bq = np.asarray(bq, dtype=np.float32) * scale
    Wk = np.asarray(Wk, dtype=np.float32)
    bk = np.asarray(bk, dtype=np.float32)

    bq_c = bq.reshape(DIM, 1).copy()
    bk_c = bk.reshape(DIM, 1).copy()

    tbl = np.asarray(rpb_table, dtype=np.float32)
    rp = np.asarray(rpi).astype(np.int64)
    bias_nmh = tbl[rp.reshape(-1)].reshape(N, N, HEADS)  # (n, m, h)
    # transposed-logits bias: rows = key m, cols = (h, n query)
    bT = bias_nmh.transpose(1, 2, 0).reshape(N, C)  # (m, (h, n))
    biasT2 = np.concatenate([bT, bT], axis=0).astype(np.float32)  # (128, C)

    # block selector: sel128[(s, m), (s', d)] = 1 if s == s'
    sel = np.zeros((128, 128), dtype=np.float16)
    sel[:64, :64] = 1.0
    sel[64:, 64:] = 1.0

    bv2 = np.tile(np.asarray(bv, dtype=np.float32)[None, :], (128, 1))
    bp2 = np.tile(np.asarray(bp, dtype=np.float32)[None, :], (128, 1))

    consts = {
        "wq": np.ascontiguousarray(Wq), "wk": np.ascontiguousarray(Wk),
        "wv": np.ascontiguousarray(np.asarray(Wv, dtype=np.float32)),
        "wp": np.ascontiguousarray(np.asarray(Wp, dtype=np.float32)),
        "biasT2": biasT2, "sel128": sel,
    }
    use_bias = bool(
        np.any(bq) or np.any(bk) or np.any(np.asarray(bv)) or np.any(np.asarray(bp))
    )
    if use_bias:
        consts.update({"bq_c": bq_c, "bk_c": bk_c, "bv2": bv2, "bp2": bp2})

    mask = np.asarray(mask, dtype=np.float32)
    use_mask = bool(np.any(mask))
    return consts, use_bias, use_mask, mask


def _maskT2_for_core(mask, w0, nw):
    """(nw//2, 128, 384): rows = (s, m key), cols = (h, n query)."""
    nwin = mask.shape[0]
    out = np.empty((nw // 2, 128, C), dtype=np.float32)
    for p in range(nw // 2):
        for s in range(2):
            w = (w0 + 2 * p + s) % nwin
            mT = mask[w].T  # (m, n)
            out[p, 64 * s : 64 * s + 64] = np.tile(mT, (1, HEADS))
    return out


_CACHE = {}


def prepare(x, cross_x, rpi, mask, Wq, bq, Wk, bk, Wv, bv, Wp, bp, rpb_table):
    """Host prep + module build; returns (nc, in_maps)."""
    x = np.ascontiguousarray(np.asarray(x, dtype=np.float32))
    cross_x = np.ascontiguousarray(np.asarray(cross_x, dtype=np.float32))
    b_ = x.shape[0]
    assert b_ % NCORES == 0
    nw = b_ // NCORES

    consts, use_bias, use_mask, mask_f = _prep_host(
        Wq, bq, Wk, bk, Wv, bv, Wp, bp, rpi, rpb_table, mask
    )

    key = (nw, use_mask, use_bias)
    if key not in _CACHE:
        _CACHE[key] = _build(nw, use_mask, use_bias)
    nc = _CACHE[key]

    in_maps = []
    for i in range(NCORES):
        m = dict(consts)
        m["x"] = x[i * nw : (i + 1) * nw]
        m["cx"] = cross_x[i * nw : (i + 1) * nw]
        if use_mask:
            m["maskT2"] = _maskT2_for_core(mask_f, i * nw, nw)
        in_maps.append(m)
    return nc, in_maps


def kernel(x, cross_x, rpi, mask, Wq, bq, Wk, bk, Wv, bv, Wp, bp, rpb_table):
    nc, in_maps = prepare(
        x, cross_x, rpi, mask, Wq, bq, Wk, bk, Wv, bv, Wp, bp, rpb_table
    )
    res = run_bass_kernel_spmd(
        nc,
        in_maps,
        core_ids=list(range(NCORES)),
        trace=bool(int(os.environ.get("KERNEL_TRACE", "0"))),
    )
    out = np.concatenate([res.results[i]["y"] for i in range(NCORES)], axis=0)
    kernel.last_exec_time_ns = res.exec_time_ns
    return out


kernel.last_exec_time_ns = None


# revision 25
# speedup vs baseline: 1.3075x; 1.3075x over previous
"""Trainium2 Bass kernel for CrossBandWindowAttention.

Reference computation (per window item b of B_=2048):
    q = (x @ Wq + bq) * scale      -> (64, 96), 6 heads x 16
    k = cross_x @ Wk + bk          -> (64, 96)
    v = cross_x @ Wv + bv          -> (64, 384), 6 heads x 64
    L_h = q_h k_h^T + rpb_bias_h (+ mask_w)
    A = softmax(L, axis=-1)
    out = (concat_h A_h v_h) @ Wp + bp
Sharding: data-parallel over b_ across 8 cores (256 windows each).

Per-core design ("transposed-logits" formulation, v2):
  - x/cross_x loaded natural, PE-transposed to channel-major xt/cxt.
  - Q/K projections emit qT/kT (96 = (head, d), tokens) in f16.
  - Q is scattered into a zero-padded block-diagonal tile q_bd so a
    single K=32 matmul per (head-pair c, window s) produces logits
    TRANSPOSED: lps rows = (s, m key-token), cols = (c, j, n query).
    6 matmuls/pair, 6-way concurrent via tile_position (3 row strips x
    2 col groups). No probability transpose is ever needed.
  - softmax: exp on ScalarE; bias folded as exp(L)*exp(bias^T) on DVE
    (f16, 2x/4x mode); row sums over the key dim (partitions) via ONE
    matmul against a constant block-selector sel128 whose (128, 128)
    structure lands the per-window sums already broadcast across all
    partitions; normalize = single DVE divide. No reduce, no transpose,
    no partition broadcast.
  - AV: 12 (64x64) matmuls, 4-way concurrent (2 row x 2 col groups),
    lhsT = v natural slices (window-major rows co-located with the
    transposed-prob rows), output channel-major = final-proj lhsT.
  - proj: 3 accumulating matmuls against Wp chunks -> (128 tokens, 384).
Engine placement: exp/og/tmp on ScalarE, bias/divide/pl on DVE,
v-copy + some transpose copies on GpSimd (otherwise idle).
Matmul operands f32r (projections) / f16 (attention core).
"""

import os
from contextlib import ExitStack

import numpy as np

import concourse.bass as bass
import concourse.mybir as mybir
import concourse.tile as tile
from concourse import bacc
from concourse.bass_utils import run_bass_kernel_spmd
from concourse.masks import make_identity

F32 = mybir.dt.float32
F32R = mybir.dt.float32r
F16 = mybir.dt.float16

DIM = 96
HEADS = 6
HD = 16  # head dim for q/k
VD = 64  # head dim for v
N = 64  # tokens per window
C = 384
NCORES = 8
B_TOTAL = 2048
GRP = 8  # windows per group (512 tokens)
TOK_G = GRP * N  # 512

EXP = mybir.ActivationFunctionType.Exp
IDENT = mybir.ActivationFunctionType.Identity
MULT = mybir.AluOpType.mult
ADD = mybir.AluOpType.add
DIVIDE = mybir.AluOpType.divide


def _build(nw, use_mask, use_bias):
    """Build the per-core Bass module for `nw` windows."""
    nc = bacc.Bacc("TRN2", target_bir_lowering=False, debug=False)

    d_x = nc.dram_tensor("x", [nw, N, C], F32, kind="ExternalInput").ap()
    d_cx = nc.dram_tensor("cx", [nw, N, C], F32, kind="ExternalInput").ap()
    d_wq = nc.dram_tensor("wq", [C, DIM], F32R, kind="ExternalInput").ap()
    d_wk = nc.dram_tensor("wk", [C, DIM], F32R, kind="ExternalInput").ap()
    d_wv = nc.dram_tensor("wv", [C, C], F32R, kind="ExternalInput").ap()
    d_wp = nc.dram_tensor("wp", [C, C], F32R, kind="ExternalInput").ap()
    d_biasT2 = nc.dram_tensor("biasT2", [128, C], F32, kind="ExternalInput").ap()
    d_sel = nc.dram_tensor("sel128", [128, 128], F16, kind="ExternalInput").ap()
    if use_bias:
        d_bq = nc.dram_tensor("bq_c", [DIM, 1], F32, kind="ExternalInput").ap()
        d_bk = nc.dram_tensor("bk_c", [DIM, 1], F32, kind="ExternalInput").ap()
        d_bv2 = nc.dram_tensor("bv2", [128, C], F32, kind="ExternalInput").ap()
        d_bp2 = nc.dram_tensor("bp2", [128, C], F32, kind="ExternalInput").ap()
    if use_mask:
        d_maskT2 = nc.dram_tensor(
            "maskT2", [nw // 2, 128, C], F32, kind="ExternalInput"
        ).ap()
    d_y = nc.dram_tensor("y", [nw, N, C], F32, kind="ExternalOutput").ap()

    x_flat = d_x.rearrange("w n c -> (w n) c")
    cx_flat = d_cx.rearrange("w n c -> (w n) c")
    y_flat = d_y.rearrange("w n c -> (w n) c")

    npair = nw // 2
    n_grp = nw // GRP

    with tile.TileContext(nc) as tc, ExitStack() as ctx:
        const = ctx.enter_context(tc.tile_pool(name="const", bufs=1))
        p_nat = ctx.enter_context(tc.tile_pool(name="p_nat", bufs=4))
        p_xt = ctx.enter_context(tc.tile_pool(name="p_xt", bufs=2))
        p_qk = ctx.enter_context(tc.tile_pool(name="p_qk", bufs=2))
        p_v = ctx.enter_context(tc.tile_pool(name="p_v", bufs=5))
        p_sm = ctx.enter_context(tc.tile_pool(name="p_sm", bufs=5))
        p_pl = ctx.enter_context(tc.tile_pool(name="p_pl", bufs=2))
        p_out = ctx.enter_context(tc.tile_pool(name="p_out", bufs=2))
        # PSUM: 4 pools x 2 bufs = 8 banks exactly.
        ps_a = ctx.enter_context(tc.tile_pool(name="ps_a", bufs=2, space="PSUM"))
        ps_vf = ctx.enter_context(tc.tile_pool(name="ps_vf", bufs=2, space="PSUM"))
        ps_l = ctx.enter_context(tc.tile_pool(name="ps_l", bufs=2, space="PSUM"))
        ps_p = ctx.enter_context(tc.tile_pool(name="ps_p", bufs=2, space="PSUM"))

        # ---- constants in SBUF ----
        ident = const.tile([128, 128], F32, name="ident")
        make_identity(nc, ident[:])

        wq_sb = const.tile([128, 3, DIM], F32R, name="wq_sb")
        wk_sb = const.tile([128, 3, DIM], F32R, name="wk_sb")
        wv_sb = const.tile([128, 3, C], F32R, name="wv_sb")
        wp_sb = const.tile([128, 3, C], F32R, name="wp_sb")
        biasT2_sb = const.tile([128, C], F32, name="biasT2_sb")
        sel_sb = const.tile([128, 128], F16, name="sel_sb")
        for Ci in range(3):
            sl = slice(128 * Ci, 128 * Ci + 128)
            nc.sync.dma_start(wq_sb[:, Ci], d_wq[sl, :])
            nc.sync.dma_start(wk_sb[:, Ci], d_wk[sl, :])
            nc.sync.dma_start(wv_sb[:, Ci], d_wv[sl, :])
            nc.sync.dma_start(wp_sb[:, Ci], d_wp[sl, :])
        nc.sync.dma_start(biasT2_sb[:], d_biasT2[:])
        nc.sync.dma_start(sel_sb[:], d_sel[:])
        expbT_sb = const.tile([128, C], F16, name="expbT_sb")
        nc.scalar.activation(expbT_sb[:], biasT2_sb[:], EXP)
        if use_bias:
            bq_sb = const.tile([DIM, 1], F32, name="bq_sb")
            bk_sb = const.tile([DIM, 1], F32, name="bk_sb")
            bv2_sb = const.tile([128, C], F32, name="bv2_sb")
            bp2_sb = const.tile([128, C], F32, name="bp2_sb")
            nc.sync.dma_start(bq_sb[:], d_bq[:])
            nc.sync.dma_start(bk_sb[:], d_bk[:])
            nc.sync.dma_start(bv2_sb[:], d_bv2[:])
            nc.sync.dma_start(bp2_sb[:], d_bp2[:])

        # q_bd: (32, 3, GRP, 128) f16, block-diagonal per head pair, all
        # chunks at partitions 0-31. Rows 16j+d = head 2c+j of chunk c;
        # cols (w, 64j + n). Two persistent tensors alternating by group
        # parity (a single one would serialize group g+1's build behind
        # group g's QK reads): off-diagonal blocks zeroed once; each
        # group's DMAs rewrite only the diagonal blocks.
        q_bds = []
        for b in range(2):
            t = const.tile([32, 3, GRP, 128], F16, name=f"q_bd{b}")
            nc.vector.memset(t[:], 0.0)
            q_bds.append(t)

        def transpose_in(src_flat, tok0, tag):
            """Load 512 tokens natural; PE-transpose to (128, 3, 512).

            PSUM tiles are packed channel-chunk-major: tile Ci holds the
            4 token-blocks of chunk Ci, so one (128, 512) copy per chunk
            lands contiguously in xt."""
            xt = p_xt.tile([128, 3, TOK_G], F32R, tag=f"xt_{tag}", name=f"xt_{tag}")
            nat = p_nat.tile([128, 4, C], F32, tag="nat", name=f"nat_{tag}")
            nc.sync.dma_start(
                nat[:],
                src_flat[tok0 : tok0 + TOK_G, :].rearrange("(t p) c -> p t c", p=128),
            )
            for Ci in range(3):
                tp = ps_a.tile([128, TOK_G], F32, tag="ps_a", name=f"tps_{tag}{Ci}")
                for t in range(4):
                    nc.tensor.transpose(
                        tp[:, 128 * t : 128 * (t + 1)],
                        nat[:, t, 128 * Ci : 128 * (Ci + 1)],
                        ident[:],
                    )
                if Ci == 2:
                    nc.scalar.copy(xt[:, Ci], tp[:])
                else:
                    nc.vector.tensor_copy(xt[:, Ci], tp[:])
            return xt

        def preamble(g):
            """Group-level: loads, transposes, Q/K projections, q_bd."""
            tok0 = g * TOK_G
            xt = transpose_in(x_flat, tok0, "x")
            cxt = transpose_in(cx_flat, tok0, "c")

            def qk_proj(src_t, w, b, tag):
                pq = ps_a.tile([DIM, TOK_G], F32, tag="ps_a", name=f"pq_{tag}")
                for Ci in range(3):
                    nc.tensor.matmul(
                        pq[:], w[:, Ci], src_t[:, Ci],
                        start=(Ci == 0), stop=(Ci == 2),
                    )
                return pq

            # tmp_q keeps the (96, T) layout (DMA source only); tmp_k is
            # split per head-pair chunk to partitions 0-31 so every QK
            # matmul sits in row strip 0 — concurrent matmuls must never
            # target the same PSUM partitions from different row groups.
            pq_q = qk_proj(xt, wq_sb, bq_sb if use_bias else None, "q")
            tmp_q = p_qk.tile([DIM, TOK_G], F16, tag="tmp_q", name="tmp_q")
            if use_bias:
                nc.scalar.activation(tmp_q[:], pq_q[:], IDENT, bias=bq_sb[:])
            else:
                nc.scalar.copy(tmp_q[:], pq_q[:])
            pq_k = qk_proj(cxt, wk_sb, bk_sb if use_bias else None, "k")
            tmp_k = p_qk.tile([32, 3, TOK_G], F16, tag="tmp_k", name="tmp_k")
            for cc in range(3):
                src = pq_k[32 * cc : 32 * cc + 32, :]
                if use_bias:
                    nc.scalar.activation(
                        tmp_k[:, cc], src, IDENT,
                        bias=bk_sb[32 * cc : 32 * cc + 32],
                    )
                elif cc == 1:
                    nc.scalar.copy(tmp_k[:, cc], src)
                else:
                    nc.vector.tensor_copy(tmp_k[:, cc], src)

            q_bd = q_bds[g % 2]
            for h in range(HEADS):
                cc, j = divmod(h, 2)
                nc.sync.dma_start(
                    q_bd[16 * j : 16 * j + 16, cc, :, 64 * j : 64 * j + 64],
                    tmp_q[16 * h : 16 * h + 16, :].rearrange(
                        "p (w n) -> p w n", n=64
                    ),
                )
            return cxt, tmp_k, q_bd

        # Software-pipelined flat loop over pairs: stage A (V-proj + QK +
        # softmax head) for pair i runs on the PE before stage B (sums +
        # AV + proj) of pair i-1, hiding the scalar/vector latency between
        # QK and the sums matmul.
        stash = {}  # pair index -> tiles needed by stage B
        group_state = None

        def stage_a(i):
            nonlocal group_state
            g, ip = divmod(i, 4)
            if ip == 0:
                group_state = preamble(g)
            cxt, tmp_k, q_bd = group_state
            ptok = 128 * ip  # pair token offset within group

            # ---- V projection (pair tokens natural) ----
            vps = ps_vf.tile([128, C], F32, tag="ps_vf", name="vps")
            for Ci in range(3):
                nc.tensor.matmul(
                    vps[:], cxt[:, Ci, ptok : ptok + 128], wv_sb[:, Ci],
                    start=(Ci == 0), stop=(Ci == 2),
                )
            # per-window v rebased to partitions 0-63 so every AV matmul
            # sits in row strips 0-1 (concurrent matmuls to the same PSUM
            # partitions must share row groups).
            v2 = p_v.tile([64, 2, C], F16, tag="vnat", name="v2")
            for s in range(2):
                sl = slice(64 * s, 64 * s + 64)
                if use_bias:
                    nc.vector.tensor_tensor(
                        v2[:, s], vps[sl, :], bv2_sb[sl, :], op=ADD
                    )
                else:
                    nc.vector.tensor_copy(v2[:, s], vps[sl, :])

            # ---- QK^T: transposed logits, 6 matmuls (2-way concurrent) ----
            # lps rows = (s, m) key tokens; cols = (c, j, n) = 128c+64j+n.
            lps = ps_l.tile([128, C], F32, tag="ps_l", name="lps")
            for cc in range(3):
                for s in range(2):
                    tok_s = ptok + 64 * s
                    nc.tensor.matmul(
                        lps[64 * s : 64 * s + 64, 128 * cc : 128 * cc + 128],
                        tmp_k[:, cc, tok_s : tok_s + 64],
                        q_bd[:, cc, 2 * ip + s, :],
                        start=True, stop=True,
                        tile_position=(0, 64 * s),
                    )

            # ---- exp + relative-position bias ----
            ee = p_sm.tile([128, C], F16, tag="ee", name="ee")
            if use_mask:
                m_sb = p_sm.tile([128, C], F32, tag="msk", name="m_sb")
                nc.sync.dma_start(m_sb[:], d_maskT2[2 * g + ip])
                e_f32 = p_sm.tile([128, C], F32, tag="ef", name="e_f32")
                nc.vector.tensor_tensor(e_f32[:], lps[:], m_sb[:], op=ADD)
                nc.scalar.activation(ee[:], e_f32[:], EXP)
            else:
                nc.scalar.activation(ee[:], lps[:], EXP)
            E = p_sm.tile([128, C], F16, tag="E", name="E")
            nc.vector.tensor_tensor(E[:], ee[:], expbT_sb[:], op=MULT)
            stash[i] = (v2, E)

        def stage_b1(i):
            v2, E = stash.pop(i)

            # ---- key-dim sums, broadcast across partitions, via PE ----
            sps = ps_l.tile([128, C], F32, tag="ps_l", name="sps")
            nc.tensor.matmul(sps[:], sel_sb[:], E[:], start=True, stop=True)
            rec = p_sm.tile([128, C], F32, tag="rec", name="rec")
            nc.vector.reciprocal_approx_fast(rec[:], sps[:])
            een = p_sm.tile([64, 2, C], F16, tag="een", name="een")
            for s in range(2):
                sl = slice(64 * s, 64 * s + 64)
                nc.vector.tensor_tensor(
                    een[:, s], E[sl, :], rec[sl, :], op=MULT
                )
            stash[(i, "b2")] = (v2, een)

        def stage_b2(i):
            g, ip = divmod(i, 4)
            v2, een = stash.pop((i, "b2"))

            # ---- AV: 12 matmuls, 2-way concurrent, channel-major out ----
            # pps rows = (j, d); cols = (c, s, n) = 128c+64s+n. All
            # operands at base 0 (row strips 0-1); consecutive matmuls
            # alternate output col groups (j) so pairs run concurrently.
            pps = ps_p.tile([128, C], F32, tag="ps_p", name="pps")
            for cc in range(3):
                for s in range(2):
                    for j in range(2):
                        h = 2 * cc + j
                        nc.tensor.matmul(
                            pps[64 * j : 64 * j + 64,
                                128 * cc + 64 * s : 128 * cc + 64 * s + 64],
                            v2[:, s, 64 * h : 64 * h + 64],
                            een[:, s,
                                128 * cc + 64 * j : 128 * cc + 64 * j + 64],
                            start=True, stop=True,
                            tile_position=(0, 64 * j),
                        )
            pl_sb = p_pl.tile([128, C], F32R, tag="pl", name="pl_sb")
            nc.scalar.copy(pl_sb[:], pps[:])
            if ip == 0:
                og = p_out.tile([128, 4, C], F32, tag="og", name="og")
                stash[("og", g)] = og
            og = stash[("og", g)]

            # ---- output projection ----
            fps = ps_vf.tile([128, C], F32, tag="ps_vf", name="fps")
            for Ci in range(3):
                nc.tensor.matmul(
                    fps[:], pl_sb[:, 128 * Ci : 128 * (Ci + 1)], wp_sb[:, Ci],
                    start=(Ci == 0), stop=(Ci == 2),
                )
            if use_bias:
                nc.scalar.activation(og[:, ip], fps[:], IDENT)
                nc.vector.tensor_tensor(og[:, ip], og[:, ip], bp2_sb[:], op=ADD)
            else:
                nc.scalar.copy(og[:, ip], fps[:])
            if ip == 3:
                stash.pop(("og", g))
                tok0 = g * TOK_G
                nc.scalar.dma_start(
                    y_flat[tok0 : tok0 + TOK_G, :].rearrange(
                        "(t p) c -> p t c", p=128
                    ),
                    og[:],
                )

        for i in range(npair):
            stage_a(i)
            if i >= 1:
                stage_b1(i - 1)
            if i >= 2:
                stage_b2(i - 2)
        stage_b1(npair - 1)
        stage_b2(npair - 2)
        stage_b2(npair - 1)

    nc.compile()
    return nc


def _prep_host(Wq, bq, Wk, bk, Wv, bv, Wp, bp, rpi, rpb_table, mask):
    scale = HD ** (-0.5)
    Wq = np.asarray(Wq, dtype=np.float32) * scale
    bq = np.asarray(bq, dtype=np.float32) * scale
    Wk = np.asarray(Wk, dtype=np.float32)
    bk = np.asarray(bk, dtype=np.float32)

    bq_c = bq.reshape(DIM, 1).copy()
    bk_c = bk.reshape(DIM, 1).copy()

    tbl = np.asarray(rpb_table, dtype=np.float32)
    rp = np.asarray(rpi).astype(np.int64)
    bias_nmh = tbl[rp.reshape(-1)].reshape(N, N, HEADS)  # (n, m, h)
    # transposed-logits bias: rows = key m, cols = (h, n query)
    bT = bias_nmh.transpose(1, 2, 0).reshape(N, C)  # (m, (h, n))
    biasT2 = np.concatenate([bT, bT], axis=0).astype(np.float32)  # (128, C)

    # block selector: sel128[(s, m), (s', d)] = 1 if s == s'
    sel = np.zeros((128, 128), dtype=np.float16)
    sel[:64, :64] = 1.0
    sel[64:, 64:] = 1.0

    bv2 = np.tile(np.asarray(bv, dtype=np.float32)[None, :], (128, 1))
    bp2 = np.tile(np.asarray(bp, dtype=np.float32)[None, :], (128, 1))

    consts = {
        "wq": np.ascontiguousarray(Wq), "wk": np.ascontiguousarray(Wk),
        "wv": np.ascontiguousarray(np.asarray(Wv, dtype=np.float32)),
        "wp": np.ascontiguousarray(np.asarray(Wp, dtype=np.float32)),
        "biasT2": biasT2, "sel128": sel,
    }
    use_bias = bool(
        np.any(bq) or np.any(bk) or np.any(np.asarray(bv)) or np.any(np.asarray(bp))
    )
    if use_bias:
        consts.update({"bq_c": bq_c, "bk_c": bk_c, "bv2": bv2, "bp2": bp2})

    mask = np.asarray(mask, dtype=np.float32)
    use_mask = bool(np.any(mask))
    return consts, use_bias, use_mask, mask


def _maskT2_for_core(mask, w0, nw):
    """(nw//2, 128, 384): rows = (s, m key), cols = (h, n query)."""
    nwin = mask.shape[0]
    out = np.empty((nw // 2, 128, C), dtype=np.float32)
    for p in range(nw // 2):
        for s in range(2):
            w = (w0 + 2 * p + s) % nwin
            mT = mask[w].T  # (m, n)
            out[p, 64 * s : 64 * s + 64] = np.tile(mT, (1, HEADS))
    return out


_CACHE = {}


def prepare(x, cross_x, rpi, mask, Wq, bq, Wk, bk, Wv, bv, Wp, bp, rpb_table):
    """Host prep + module build; returns (nc, in_maps)."""
    x = np.ascontiguousarray(np.asarray(x, dtype=np.float32))
    cross_x = np.ascontiguousarray(np.asarray(cross_x, dtype=np.float32))
    b_ = x.shape[0]
    assert b_ % NCORES == 0
    nw = b_ // NCORES

    consts, use_bias, use_mask, mask_f = _prep_host(
        Wq, bq, Wk, bk, Wv, bv, Wp, bp, rpi, rpb_table, mask
    )

    key = (nw, use_mask, use_bias)
    if key not in _CACHE:
        _CACHE[key] = _build(nw, use_mask, use_bias)
    nc = _CACHE[key]

    in_maps = []
    for i in range(NCORES):
        m = dict(consts)
        m["x"] = x[i * nw : (i + 1) * nw]
        m["cx"] = cross_x[i * nw : (i + 1) * nw]
        if use_mask:
            m["maskT2"] = _maskT2_for_core(mask_f, i * nw, nw)
        in_maps.append(m)
    return nc, in_maps


def kernel(x, cross_x, rpi, mask, Wq, bq, Wk, bk, Wv, bv, Wp, bp, rpb_table):
    nc, in_maps = prepare(
        x, cross_x, rpi, mask, Wq, bq, Wk, bk, Wv, bv, Wp, bp, rpb_table
    )
    res = run_bass_kernel_spmd(
        nc,
        in_maps,
        core_ids=list(range(NCORES)),
        trace=bool(int(os.environ.get("KERNEL_TRACE", "0"))),
    )
    out = np.concatenate([res.results[i]["y"] for i in range(NCORES)], axis=0)
    kernel.last_exec_time_ns = res.exec_time_ns
    return out


kernel.last_exec_time_ns = None


# revision 26
# speedup vs baseline: 1.4221x; 1.0877x over previous
"""Trainium2 Bass kernel for CrossBandWindowAttention.

Reference computation (per window item b of B_=2048):
    q = (x @ Wq + bq) * scale      -> (64, 96), 6 heads x 16
    k = cross_x @ Wk + bk          -> (64, 96)
    v = cross_x @ Wv + bv          -> (64, 384), 6 heads x 64
    L_h = q_h k_h^T + rpb_bias_h (+ mask_w)
    A = softmax(L, axis=-1)
    out = (concat_h A_h v_h) @ Wp + bp
Sharding: data-parallel over b_ across 8 cores (256 windows each).

Per-core design ("transposed-logits" formulation, v2):
  - x/cross_x loaded natural, PE-transposed to channel-major xt/cxt.
  - Q/K projections emit qT/kT (96 = (head, d), tokens) in f16.
  - Q is scattered into a zero-padded block-diagonal tile q_bd so a
    single K=32 matmul per (head-pair c, window s) produces logits
    TRANSPOSED: lps rows = (s, m key-token), cols = (c, j, n query).
    6 matmuls/pair, 6-way concurrent via tile_position (3 row strips x
    2 col groups). No probability transpose is ever needed.
  - softmax: exp on ScalarE; bias folded as exp(L)*exp(bias^T) on DVE
    (f16, 2x/4x mode); row sums over the key dim (partitions) via ONE
    matmul against a constant block-selector sel128 whose (128, 128)
    structure lands the per-window sums already broadcast across all
    partitions; normalize = single DVE divide. No reduce, no transpose,
    no partition broadcast.
  - AV: 12 (64x64) matmuls, 4-way concurrent (2 row x 2 col groups),
    lhsT = v natural slices (window-major rows co-located with the
    transposed-prob rows), output channel-major = final-proj lhsT.
  - proj: 3 accumulating matmuls against Wp chunks -> (128 tokens, 384).
Engine placement: exp/og/tmp on ScalarE, bias/divide/pl on DVE,
v-copy + some transpose copies on GpSimd (otherwise idle).
Matmul operands f32r (projections) / f16 (attention core).
"""

import os
from contextlib import ExitStack

import numpy as np

import concourse.bass as bass
import concourse.mybir as mybir
import concourse.tile as tile
from concourse import bacc
from concourse.bass_utils import run_bass_kernel_spmd
from concourse.masks import make_identity

F32 = mybir.dt.float32
F32R = mybir.dt.float32r
F16 = mybir.dt.float16

DIM = 96
HEADS = 6
HD = 16  # head dim for q/k
VD = 64  # head dim for v
N = 64  # tokens per window
C = 384
NCORES = 8
B_TOTAL = 2048
GRP = 8  # windows per group (512 tokens)
TOK_G = GRP * N  # 512

EXP = mybir.ActivationFunctionType.Exp
IDENT = mybir.ActivationFunctionType.Identity
MULT = mybir.AluOpType.mult
ADD = mybir.AluOpType.add
DIVIDE = mybir.AluOpType.divide


def _build(nw, use_mask, use_bias):
    """Build the per-core Bass module for `nw` windows."""
    nc = bacc.Bacc("TRN2", target_bir_lowering=False, debug=False)

    d_x = nc.dram_tensor("x", [nw, N, C], F32, kind="ExternalInput").ap()
    d_cx = nc.dram_tensor("cx", [nw, N, C], F32, kind="ExternalInput").ap()
    d_wq = nc.dram_tensor("wq", [C, DIM], F32R, kind="ExternalInput").ap()
    d_wk = nc.dram_tensor("wk", [C, DIM], F32R, kind="ExternalInput").ap()
    d_wv = nc.dram_tensor("wv", [C, C], F32R, kind="ExternalInput").ap()
    d_wp = nc.dram_tensor("wp", [C, C], F32R, kind="ExternalInput").ap()
    d_biasT2 = nc.dram_tensor("biasT2", [128, C], F32, kind="ExternalInput").ap()
    d_sel = nc.dram_tensor("sel128", [128, 128], F16, kind="ExternalInput").ap()
    if use_bias:
        d_bq = nc.dram_tensor("bq_c", [DIM, 1], F32, kind="ExternalInput").ap()
        d_bk = nc.dram_tensor("bk_c", [DIM, 1], F32, kind="ExternalInput").ap()
        d_bv2 = nc.dram_tensor("bv2", [128, C], F32, kind="ExternalInput").ap()
        d_bp2 = nc.dram_tensor("bp2", [128, C], F32, kind="ExternalInput").ap()
    if use_mask:
        d_maskT2 = nc.dram_tensor(
            "maskT2", [nw // 2, 128, C], F32, kind="ExternalInput"
        ).ap()
    d_y = nc.dram_tensor("y", [nw, N, C], F32, kind="ExternalOutput").ap()

    x_flat = d_x.rearrange("w n c -> (w n) c")
    cx_flat = d_cx.rearrange("w n c -> (w n) c")
    y_flat = d_y.rearrange("w n c -> (w n) c")

    npair = nw // 2
    n_grp = nw // GRP

    with tile.TileContext(nc) as tc, ExitStack() as ctx:
        const = ctx.enter_context(tc.tile_pool(name="const", bufs=1))
        p_nat = ctx.enter_context(tc.tile_pool(name="p_nat", bufs=4))
        p_xt = ctx.enter_context(tc.tile_pool(name="p_xt", bufs=2))
        p_qk = ctx.enter_context(tc.tile_pool(name="p_qk", bufs=2))
        p_v = ctx.enter_context(tc.tile_pool(name="p_v", bufs=5))
        p_sm = ctx.enter_context(tc.tile_pool(name="p_sm", bufs=5))
        p_pl = ctx.enter_context(tc.tile_pool(name="p_pl", bufs=2))
        p_out = ctx.enter_context(tc.tile_pool(name="p_out", bufs=2))
        # PSUM: 4 pools x 2 bufs = 8 banks exactly.
        ps_a = ctx.enter_context(tc.tile_pool(name="ps_a", bufs=2, space="PSUM"))
        ps_vf = ctx.enter_context(tc.tile_pool(name="ps_vf", bufs=2, space="PSUM"))
        ps_l = ctx.enter_context(tc.tile_pool(name="ps_l", bufs=2, space="PSUM"))
        ps_p = ctx.enter_context(tc.tile_pool(name="ps_p", bufs=2, space="PSUM"))

        # ---- constants in SBUF ----
        ident = const.tile([128, 128], F32, name="ident")
        make_identity(nc, ident[:])

        wq_sb = const.tile([128, 3, DIM], F32R, name="wq_sb")
        wk_sb = const.tile([128, 3, DIM], F32R, name="wk_sb")
        wv_sb = const.tile([128, 3, C], F32R, name="wv_sb")
        wp_sb = const.tile([128, 3, C], F32R, name="wp_sb")
        biasT2_sb = const.tile([128, C], F32, name="biasT2_sb")
        sel_sb = const.tile([128, 128], F16, name="sel_sb")
        for Ci in range(3):
            sl = slice(128 * Ci, 128 * Ci + 128)
            nc.sync.dma_start(wq_sb[:, Ci], d_wq[sl, :])
            nc.sync.dma_start(wk_sb[:, Ci], d_wk[sl, :])
            nc.sync.dma_start(wv_sb[:, Ci], d_wv[sl, :])
            nc.sync.dma_start(wp_sb[:, Ci], d_wp[sl, :])
        nc.sync.dma_start(biasT2_sb[:], d_biasT2[:])
        nc.sync.dma_start(sel_sb[:], d_sel[:])
        expbT_sb = const.tile([128, C], F16, name="expbT_sb")
        nc.scalar.activation(expbT_sb[:], biasT2_sb[:], EXP)
        if use_bias:
            bq_sb = const.tile([DIM, 1], F32, name="bq_sb")
            bk_sb = const.tile([DIM, 1], F32, name="bk_sb")
            bv2_sb = const.tile([128, C], F32, name="bv2_sb")
            bp2_sb = const.tile([128, C], F32, name="bp2_sb")
            nc.sync.dma_start(bq_sb[:], d_bq[:])
            nc.sync.dma_start(bk_sb[:], d_bk[:])
            nc.sync.dma_start(bv2_sb[:], d_bv2[:])
            nc.sync.dma_start(bp2_sb[:], d_bp2[:])

        # q_bd: (32, 3, GRP, 128) f16, block-diagonal per head pair, all
        # chunks at partitions 0-31. Rows 16j+d = head 2c+j of chunk c;
        # cols (w, 64j + n). Two persistent tensors alternating by group
        # parity (a single one would serialize group g+1's build behind
        # group g's QK reads): off-diagonal blocks zeroed once; each
        # group's DMAs rewrite only the diagonal blocks.
        q_bds = []
        for b in range(2):
            t = const.tile([32, 3, GRP, 128], F16, name=f"q_bd{b}")
            nc.vector.memset(t[:], 0.0)
            q_bds.append(t)

        def load_nat(src_flat, tok0, tag):
            nat = p_nat.tile([128, 4, C], F32, tag="nat", name=f"nat_{tag}")
            nc.sync.dma_start(
                nat[:],
                src_flat[tok0 : tok0 + TOK_G, :].rearrange("(t p) c -> p t c", p=128),
            )
            return nat

        def transpose_in(nat, tag):
            """PE-transpose a preloaded natural tile to (128, 3, 512).

            PSUM tiles are packed channel-chunk-major: tile Ci holds the
            4 token-blocks of chunk Ci, so one (128, 512) copy per chunk
            lands contiguously in xt."""
            xt = p_xt.tile([128, 3, TOK_G], F32R, tag=f"xt_{tag}", name=f"xt_{tag}")
            for Ci in range(3):
                tp = ps_a.tile([128, TOK_G], F32, tag="ps_a", name=f"tps_{tag}{Ci}")
                for t in range(4):
                    nc.tensor.transpose(
                        tp[:, 128 * t : 128 * (t + 1)],
                        nat[:, t, 128 * Ci : 128 * (Ci + 1)],
                        ident[:],
                    )
                if Ci == 2:
                    nc.scalar.copy(xt[:, Ci], tp[:])
                else:
                    nc.vector.tensor_copy(xt[:, Ci], tp[:])
            return xt

        def qk_proj(src_t, w, tag):
            pq = ps_a.tile([DIM, TOK_G], F32, tag="ps_a", name=f"pq_{tag}")
            for Ci in range(3):
                nc.tensor.matmul(
                    pq[:], w[:, Ci], src_t[:, Ci],
                    start=(Ci == 0), stop=(Ci == 2),
                )
            return pq

        def qk_projs(xt, cxt):
            # tmp_q keeps the (96, T) layout (DMA source only); tmp_k is
            # split per head-pair chunk to partitions 0-31 so every QK
            # matmul sits in row strip 0 — concurrent matmuls must never
            # target the same PSUM partitions from different row groups.
            pq_q = qk_proj(xt, wq_sb, "q")
            tmp_q = p_qk.tile([DIM, TOK_G], F16, tag="tmp_q", name="tmp_q")
            if use_bias:
                nc.scalar.activation(tmp_q[:], pq_q[:], IDENT, bias=bq_sb[:])
            else:
                nc.scalar.copy(tmp_q[:], pq_q[:])
            pq_k = qk_proj(cxt, wk_sb, "k")
            tmp_k = p_qk.tile([32, 3, TOK_G], F16, tag="tmp_k", name="tmp_k")
            for cc in range(3):
                src = pq_k[32 * cc : 32 * cc + 32, :]
                if use_bias:
                    nc.scalar.activation(
                        tmp_k[:, cc], src, IDENT,
                        bias=bk_sb[32 * cc : 32 * cc + 32],
                    )
                elif cc == 1:
                    nc.scalar.copy(tmp_k[:, cc], src)
                else:
                    nc.vector.tensor_copy(tmp_k[:, cc], src)
            return tmp_q, tmp_k

        def build_q_bd(g, tmp_q):
            q_bd = q_bds[g % 2]
            for h in range(HEADS):
                cc, j = divmod(h, 2)
                nc.sync.dma_start(
                    q_bd[16 * j : 16 * j + 16, cc, :, 64 * j : 64 * j + 64],
                    tmp_q[16 * h : 16 * h + 16, :].rearrange(
                        "p (w n) -> p w n", n=64
                    ),
                )
            return q_bd

        # Software-pipelined flat loop over pairs: stage A (V-proj + QK +
        # softmax head) for pair i runs on the PE before stage B (sums +
        # AV + proj) of pair i-1, hiding the scalar/vector latency between
        # QK and the sums matmul. The NEXT group's preamble is spread
        # piecewise across the current group's four iterations so the PE
        # transpose bursts stay interleaved with matmul activity (PE
        # transposes don't count as busy for the HAM clock monitor).
        stash = {}  # pair index -> tiles needed by stage B
        group_state = None
        next_state = {}

        def pre_part(g, k):
            """Emit part k (0..3) of group g's preamble."""
            st = next_state
            tok0 = g * TOK_G
            if k == 0:
                st["nat_x"] = load_nat(x_flat, tok0, "x")
                st["nat_c"] = load_nat(cx_flat, tok0, "c")
                st["xt"] = transpose_in(st["nat_x"], "x")
            elif k == 1:
                st["cxt"] = transpose_in(st["nat_c"], "c")
            elif k == 2:
                st["qk"] = qk_projs(st["xt"], st["cxt"])
            else:
                tmp_q, tmp_k = st["qk"]
                q_bd = build_q_bd(g, tmp_q)
                st["done"] = (st["cxt"], tmp_k, q_bd)

        def stage_a(i):
            nonlocal group_state
            g, ip = divmod(i, 4)
            if ip == 0:
                if g == 0:
                    for k in range(4):
                        pre_part(0, k)
                group_state = next_state.pop("done")
                next_state.clear()
            if g + 1 < n_grp:
                pre_part(g + 1, ip)
            cxt, tmp_k, q_bd = group_state
            ptok = 128 * ip  # pair token offset within group

            # ---- V projection (pair tokens natural) ----
            vps = ps_vf.tile([128, C], F32, tag="ps_vf", name="vps")
            for Ci in range(3):
                nc.tensor.matmul(
                    vps[:], cxt[:, Ci, ptok : ptok + 128], wv_sb[:, Ci],
                    start=(Ci == 0), stop=(Ci == 2),
                )
            # per-window v rebased to partitions 0-63 so every AV matmul
            # sits in row strips 0-1 (concurrent matmuls to the same PSUM
            # partitions must share row groups).
            v2 = p_v.tile([64, 2, C], F16, tag="vnat", name="v2")
            for s in range(2):
                sl = slice(64 * s, 64 * s + 64)
                if use_bias:
                    nc.vector.tensor_tensor(
                        v2[:, s], vps[sl, :], bv2_sb[sl, :], op=ADD
                    )
                else:
                    nc.vector.tensor_copy(v2[:, s], vps[sl, :])

            # ---- QK^T: transposed logits, 6 matmuls (2-way concurrent) ----
            # lps rows = (s, m) key tokens; cols = (c, j, n) = 128c+64j+n.
            lps = ps_l.tile([128, C], F32, tag="ps_l", name="lps")
            for cc in range(3):
                for s in range(2):
                    tok_s = ptok + 64 * s
                    nc.tensor.matmul(
                        lps[64 * s : 64 * s + 64, 128 * cc : 128 * cc + 128],
                        tmp_k[:, cc, tok_s : tok_s + 64],
                        q_bd[:, cc, 2 * ip + s, :],
                        start=True, stop=True,
                        tile_position=(0, 64 * s),
                    )

            # ---- exp + relative-position bias ----
            ee = p_sm.tile([128, C], F16, tag="ee", name="ee")
            if use_mask:
                m_sb = p_sm.tile([128, C], F32, tag="msk", name="m_sb")
                nc.sync.dma_start(m_sb[:], d_maskT2[2 * g + ip])
                e_f32 = p_sm.tile([128, C], F32, tag="ef", name="e_f32")
                nc.vector.tensor_tensor(e_f32[:], lps[:], m_sb[:], op=ADD)
                nc.scalar.activation(ee[:], e_f32[:], EXP)
            else:
                nc.scalar.activation(ee[:], lps[:], EXP)
            E = p_sm.tile([128, C], F16, tag="E", name="E")
            nc.vector.tensor_tensor(E[:], ee[:], expbT_sb[:], op=MULT)
            stash[i] = (v2, E)

        def stage_b1(i):
            v2, E = stash.pop(i)

            # ---- key-dim sums, broadcast across partitions, via PE ----
            sps = ps_l.tile([128, C], F32, tag="ps_l", name="sps")
            nc.tensor.matmul(sps[:], sel_sb[:], E[:], start=True, stop=True)
            rec = p_sm.tile([128, C], F32, tag="rec", name="rec")
            nc.vector.reciprocal_approx_fast(rec[:], sps[:])
            een = p_sm.tile([64, 2, C], F16, tag="een", name="een")
            for s in range(2):
                sl = slice(64 * s, 64 * s + 64)
                nc.vector.tensor_tensor(
                    een[:, s], E[sl, :], rec[sl, :], op=MULT
                )
            stash[(i, "b2")] = (v2, een)

        def stage_b2(i):
            g, ip = divmod(i, 4)
            v2, een = stash.pop((i, "b2"))

            # ---- AV: 12 matmuls, 2-way concurrent, channel-major out ----
            # pps rows = (j, d); cols = (c, s, n) = 128c+64s+n. All
            # operands at base 0 (row strips 0-1); consecutive matmuls
            # alternate output col groups (j) so pairs run concurrently.
            pps = ps_p.tile([128, C], F32, tag="ps_p", name="pps")
            for cc in range(3):
                for s in range(2):
                    for j in range(2):
                        h = 2 * cc + j
                        nc.tensor.matmul(
                            pps[64 * j : 64 * j + 64,
                                128 * cc + 64 * s : 128 * cc + 64 * s + 64],
                            v2[:, s, 64 * h : 64 * h + 64],
                            een[:, s,
                                128 * cc + 64 * j : 128 * cc + 64 * j + 64],
                            start=True, stop=True,
                            tile_position=(0, 64 * j),
                        )
            pl_sb = p_pl.tile([128, C], F32R, tag="pl", name="pl_sb")
            nc.scalar.copy(pl_sb[:], pps[:])
            if ip == 0:
                og = p_out.tile([128, 4, C], F32, tag="og", name="og")
                stash[("og", g)] = og
            og = stash[("og", g)]

            # ---- output projection ----
            fps = ps_vf.tile([128, C], F32, tag="ps_vf", name="fps")
            for Ci in range(3):
                nc.tensor.matmul(
                    fps[:], pl_sb[:, 128 * Ci : 128 * (Ci + 1)], wp_sb[:, Ci],
                    start=(Ci == 0), stop=(Ci == 2),
                )
            if use_bias:
                nc.scalar.activation(og[:, ip], fps[:], IDENT)
                nc.vector.tensor_tensor(og[:, ip], og[:, ip], bp2_sb[:], op=ADD)
            else:
                nc.scalar.copy(og[:, ip], fps[:])
            if ip == 3:
                stash.pop(("og", g))
                tok0 = g * TOK_G
                nc.scalar.dma_start(
                    y_flat[tok0 : tok0 + TOK_G, :].rearrange(
                        "(t p) c -> p t c", p=128
                    ),
                    og[:],
                )

        for i in range(npair):
            stage_a(i)
            if i >= 1:
                stage_b1(i - 1)
            if i >= 2:
                stage_b2(i - 2)
        stage_b1(npair - 1)
        stage_b2(npair - 2)
        stage_b2(npair - 1)

    nc.compile()
    return nc


def _prep_host(Wq, bq, Wk, bk, Wv, bv, Wp, bp, rpi, rpb_table, mask):
    scale = HD ** (-0.5)
    Wq = np.asarray(Wq, dtype=np.float32) * scale
    bq = np.asarray(bq, dtype=np.float32) * scale
    Wk = np.asarray(Wk, dtype=np.float32)
    bk = np.asarray(bk, dtype=np.float32)

    bq_c = bq.reshape(DIM, 1).copy()
    bk_c = bk.reshape(DIM, 1).copy()

    tbl = np.asarray(rpb_table, dtype=np.float32)
    rp = np.asarray(rpi).astype(np.int64)
    bias_nmh = tbl[rp.reshape(-1)].reshape(N, N, HEADS)  # (n, m, h)
    # transposed-logits bias: rows = key m, cols = (h, n query)
    bT = bias_nmh.transpose(1, 2, 0).reshape(N, C)  # (m, (h, n))
    biasT2 = np.concatenate([bT, bT], axis=0).astype(np.float32)  # (128, C)

    # block selector: sel128[(s, m), (s', d)] = 1 if s == s'
    sel = np.zeros((128, 128), dtype=np.float16)
    sel[:64, :64] = 1.0
    sel[64:, 64:] = 1.0

    bv2 = np.tile(np.asarray(bv, dtype=np.float32)[None, :], (128, 1))
    bp2 = np.tile(np.asarray(bp, dtype=np.float32)[None, :], (128, 1))

    consts = {
        "wq": np.ascontiguousarray(Wq), "wk": np.ascontiguousarray(Wk),
        "wv": np.ascontiguousarray(np.asarray(Wv, dtype=np.float32)),
        "wp": np.ascontiguousarray(np.asarray(Wp, dtype=np.float32)),
        "biasT2": biasT2, "sel128": sel,
    }
    use_bias = bool(
        np.any(bq) or np.any(bk) or np.any(np.asarray(bv)) or np.any(np.asarray(bp))
    )
    if use_bias:
        consts.update({"bq_c": bq_c, "bk_c": bk_c, "bv2": bv2, "bp2": bp2})

    mask = np.asarray(mask, dtype=np.float32)
    use_mask = bool(np.any(mask))
    return consts, use_bias, use_mask, mask


def _maskT2_for_core(mask, w0, nw):
    """(nw//2, 128, 384): rows = (s, m key), cols = (h, n query)."""
    nwin = mask.shape[0]
    out = np.empty((nw // 2, 128, C), dtype=np.float32)
    for p in range(nw // 2):
        for s in range(2):
            w = (w0 + 2 * p + s) % nwin
            mT = mask[w].T  # (m, n)
            out[p, 64 * s : 64 * s + 64] = np.tile(mT, (1, HEADS))
    return out


_CACHE = {}


def prepare(x, cross_x, rpi, mask, Wq, bq, Wk, bk, Wv, bv, Wp, bp, rpb_table):
    """Host prep + module build; returns (nc, in_maps)."""
    x = np.ascontiguousarray(np.asarray(x, dtype=np.float32))
    cross_x = np.ascontiguousarray(np.asarray(cross_x, dtype=np.float32))
    b_ = x.shape[0]
    assert b_ % NCORES == 0
    nw = b_ // NCORES

    consts, use_bias, use_mask, mask_f = _prep_host(
        Wq, bq, Wk, bk, Wv, bv, Wp, bp, rpi, rpb_table, mask
    )

    key = (nw, use_mask, use_bias)
    if key not in _CACHE:
        _CACHE[key] = _build(nw, use_mask, use_bias)
    nc = _CACHE[key]

    in_maps = []
    for i in range(NCORES):
        m = dict(consts)
        m["x"] = x[i * nw : (i + 1) * nw]
        m["cx"] = cross_x[i * nw : (i + 1) * nw]
        if use_mask:
            m["maskT2"] = _maskT2_for_core(mask_f, i * nw, nw)
        in_maps.append(m)
    return nc, in_maps


def kernel(x, cross_x, rpi, mask, Wq, bq, Wk, bk, Wv, bv, Wp, bp, rpb_table):
    nc, in_maps = prepare(
        x, cross_x, rpi, mask, Wq, bq, Wk, bk, Wv, bv, Wp, bp, rpb_table
    )
    res = run_bass_kernel_spmd(
        nc,
        in_maps,
        core_ids=list(range(NCORES)),
        trace=bool(int(os.environ.get("KERNEL_TRACE", "0"))),
    )
    out = np.concatenate([res.results[i]["y"] for i in range(NCORES)], axis=0)
    kernel.last_exec_time_ns = res.exec_time_ns
    return out


kernel.last_exec_time_ns = None
